# revision 17
# baseline (speedup 1.0000x reference)
"""Trainium2 Bass kernel for nn_MultiHeadAttention_52398601011223.

Full-input contract: kernel(**inputs) takes the complete tensors from
setup_inputs() and returns the full [4, 2048, 768] float32 output.

Sharding: 8 cores = batch(4) x query-half(2). Each core handles all 12
heads for 1024 queries of one batch, with all 2048 keys. No collectives:
each core owns its output rows end-to-end (k/v projections are computed
redundantly by the two cores sharing a batch).

Layout strategy (everything lands in its matmul-natural layout):
  - host pre-transposes Q/K/V to [768, seq] and pre-transposes the
    attention mask to a bf16 keep-mask [keys, queries]
  - projections produce qT/kT as [feature, token] (feature on partitions,
    2 heads per 128-partition block) and v as [token, feature]
  - scores are computed transposed, S^T[k, q], two heads row-packed in the
    128x128 array (d_k = 64); the pair runs concurrently via row tiling
  - exp on ScalarE (PSUM fp32 -> SBUF bf16), keep-mask applied
    multiplicatively on VectorE after exp (exp(-1e9) == 0 in the reference)
  - context uses lhsT = [V_head | ones] (65 columns): one accumulation
    yields both context^T and the softmax row-sums
  - row-sums are DMA-gathered straight out of the context PSUM rows and
    inverted with a few consolidated reciprocal_approx_fast ops
  - fc consumes context^T directly; residual + LayerNorm are per-token with
    d_model on the free axis

Scheduling: the exp stream on ScalarE (192 x ~1.1us) and the matmul
stream on TensorE are co-critical.  The front is restructured so the
first exp issues as early as possible (chunked weight DMAs so fb0 of the
q/k projections can start immediately); the q projection for fb1-5, the
whole v projection, the k projection for fb1-5 and the first fc partials
are all interleaved into the attention streams right after each score
matmul; VectorE work is trimmed (consolidated approx reciprocals, 2-op
mask multiply) to keep the scores->exp->mask->context chain tight.
"""

import numpy as np
import ml_dtypes

import concourse.bass as bass
import concourse.mybir as mybir
import concourse.tile as tile
import bass_rust
from concourse.bass_utils import run_bass_kernel_spmd

F32 = mybir.dt.float32
BF16 = mybir.dt.bfloat16
AF = mybir.ActivationFunctionType
ALU = mybir.AluOpType

B, S, DM = 4, 2048, 768
H, DK, DV = 12, 64, 64
SQ = S // 2          # queries per core
KB = S // 128        # key blocks (16)
FB = DM // 128       # feature blocks (6)
QT = SQ // 512       # 512-wide query tiles (2)
NQT = SQ // 128      # 128-row query tiles for fc/LN (8)
SCALE = 1.0 / 8.0    # 1/sqrt(d_k)
LN_EPS = 1e-5
VS = 66              # per-head stride in the v+ones sbuf layout


def _split_excess_waits(nc, maxw=1):
    """walrus CoreV3 in this build accepts only one sem-wait per
    instruction; move extras onto injected NoOps just before the owner."""
    n_new = 0
    for bb in nc.main_func.blocks:
        insts = bb.instructions  # live list
        i = 0
        while i < len(insts):
            ins = insts[i]
            si = getattr(ins, "sync_info", None)
            if si is None:
                i += 1
                continue
            waits = list(si.on_wait or [])
            if len(waits) > maxw:
                si.on_wait = waits[-maxw:]
                extra = waits[:-maxw]
                pos = i
                for j in range(0, len(extra), maxw):
                    nop = mybir.InstNoOp(name=f"waitsplit{n_new}", ins=[], outs=[])
                    n_new += 1
                    nop.engine = ins.engine
                    nop.sync_info = bass_rust.SyncInfo(
                        on_wait=extra[j : j + maxw], on_update=[]
                    )
                    insts.insert(pos, nop)
                    pos += 1
                    i += 1
            i += 1
    return n_new


def _bcast_ap(ap, nparts):
    """Partition-broadcast read AP over a [1, N] slice."""
    return bass.AP(tensor=ap.tensor, offset=ap.offset, ap=[[0, nparts]] + list(ap.ap[1:]))


def build_nc():
    nc = bass.Bass("TRN2", target_bir_lowering=False, debug=False, num_devices=8)

    qT_d = nc.dram_tensor("qT", [DM, SQ], BF16, kind="ExternalInput")
    kT_d = nc.dram_tensor("kT", [DM, S], BF16, kind="ExternalInput")
    vT_d = nc.dram_tensor("vT", [DM, S], BF16, kind="ExternalInput")
    maskT_d = nc.dram_tensor("maskT", [S, SQ], BF16, kind="ExternalInput")
    wq_d = nc.dram_tensor("wq", [DM, DM], BF16, kind="ExternalInput")
    wk_d = nc.dram_tensor("wk", [DM, DM], BF16, kind="ExternalInput")
    wv_d = nc.dram_tensor("wv", [DM, DM], BF16, kind="ExternalInput")
    wfc_d = nc.dram_tensor("wfc", [DM, DM], BF16, kind="ExternalInput")
    qres_d = nc.dram_tensor("qres", [SQ, DM], F32, kind="ExternalInput")
    out_d = nc.dram_tensor("out", [SQ, DM], F32, kind="ExternalOutput")

    wq_r = wq_d.ap().rearrange("(a p) f -> p a f", p=128)
    wk_r = wk_d.ap().rearrange("(a p) f -> p a f", p=128)
    mask_r = maskT_d.ap().rearrange("(a p) q -> p a q", p=128)

    with tile.TileContext(nc) as tc:
        with (
            tc.tile_pool(name="consts", bufs=1) as consts,
            tc.tile_pool(name="proj", bufs=1) as proj,
            tc.tile_pool(name="mm", bufs=2, space="PSUM") as mmp,
            tc.tile_pool(name="pp", bufs=1, space="PSUM") as ppp,
            tc.tile_pool(name="ctx", bufs=2, space="PSUM") as ctxp,
            tc.tile_pool(name="dram", bufs=2, space="DRAM") as dramp,
        ):
            # ---- persistent tiles -------------------------------------------
            wfc_sb = consts.tile([128, FB, DM], BF16, tag="wfc")
            mask_sb = consts.tile([128, KB, SQ], BF16, tag="mask")
            epsb = consts.tile([128, 1], F32, tag="epsb")
            nc.vector.memset(epsb[:], LN_EPS)
            warm = consts.tile([1, 2], F32, tag="warm")
            nc.vector.memset(warm[:], 1.0)
            # prewarm the exp table set while the front DMAs run
            nc.scalar.activation(warm[:, 1:2], warm[:, 0:1], AF.Exp)

            qp_sb = proj.tile([128, FB, SQ], BF16, tag="qp")
            kp_sb = proj.tile([128, FB, S], BF16, tag="kp")
            vS_sb = proj.tile([128, KB, H * VS], BF16, tag="vS")
            ctxS_sb = proj.tile([128, FB, SQ], BF16, tag="ctxS")
            # row-sum gather (bf16 rows lifted off the ctx evacuations),
            # fp32 staging for the consolidated approx reciprocals.  DVE
            # partition bases must be 32-aligned, so the 3 reciprocal
            # groups (heads 0-5 / 6-9 / 10-11) live at partitions 0/32/64.
            rsF_sb = proj.tile([128, 512], F32, tag="rsF")
            rsR_sb = proj.tile([128, 512], F32, tag="rsR")
            rsB_sb = proj.tile([128, SQ], BF16, tag="rsB")
            rrecip_dt = dramp.tile([H, SQ], BF16, tag="rrecip")

            vS3 = vS_sb.rearrange("p b (h c) -> p b h c", c=VS)
            for tb in range(KB):
                nc.vector.memset(vS3[:, tb, :, 64:66], 1.0)

            # ---- pools (alloc order = reverse of release order: LIFO) ------
            inB = tc.alloc_tile_pool(name="inB", bufs=1)
            wk_sb = inB.tile([128, FB, DM], BF16, tag="wk")
            kin_sb = inB.tile([128, FB, S], BF16, tag="kin")
            ptp = tc.alloc_tile_pool(name="pt", bufs=5)
            rsp = tc.alloc_tile_pool(name="rs", bufs=2)
            inA = tc.alloc_tile_pool(name="inA", bufs=1)
            wq_sb = inA.tile([128, FB, DM], BF16, tag="wq")
            qin_sb = inA.tile([128, FB, SQ], BF16, tag="qin")
            inC = tc.alloc_tile_pool(name="inC", bufs=1)
            wv_sb = inC.tile([128, FB, DM], BF16, tag="wv")
            vchp = tc.alloc_tile_pool(name="vch", bufs=3)

            # ---- front DMAs: ordered so fb0 q/k-proj can start ASAP ---------
            vT_r = vT_d.ap().rearrange("(a p) t -> p a t", p=128)
            vch_tiles = {}

            def vch_dma(c):
                vch = vchp.tile([128, FB, 256], BF16, tag="vch", name=f"vch{c}")
                vch_tiles[c] = vch
                nc.sync.dma_start(out=vch[:], in_=vT_r[:, :, c * 256 : (c + 1) * 256])

            # minimal set gating the first score matmul, then the rest
            nc.sync.dma_start(out=wq_sb[:, :, 0:128], in_=wq_r[:, :, 0:128])
            nc.sync.dma_start(out=qin_sb[:], in_=qT_d.ap().rearrange("(a p) t -> p a t", p=128))
            nc.sync.dma_start(out=wk_sb[:, :, 0:128], in_=wk_r[:, :, 0:128])
            nc.sync.dma_start(out=kin_sb[:], in_=kT_d.ap().rearrange("(a p) t -> p a t", p=128))
            nc.sync.dma_start(out=mask_sb[:, 0:4, :], in_=mask_r[:, 0:4, :])
            nc.sync.dma_start(out=wv_sb[:], in_=wv_d.ap().rearrange("(a p) f -> p a f", p=128))
            vch_dma(0)
            vch_dma(1)
            nc.sync.dma_start(out=mask_sb[:, 4:8, :], in_=mask_r[:, 4:8, :])
            nc.sync.dma_start(out=wq_sb[:, :, 128:768], in_=wq_r[:, :, 128:768])
            vch_dma(2)
            vch_dma(3)
            nc.sync.dma_start(out=wk_sb[:, :, 128:768], in_=wk_r[:, :, 128:768])
            nc.sync.dma_start(out=mask_sb[:, 8:KB, :], in_=mask_r[:, 8:KB, :])
            nc.sync.dma_start(out=wfc_sb[:], in_=wfc_d.ap().rearrange("(a p) f -> p a f", p=128))

            # ---- q projection: fb0 up front (ScalarE evac: ACT is idle), ----
            # fb1-5 interleaved into the attention streams as whole blocks
            def qproj(fb, evac_engine):
                ps = mmp.tile([128, 1024], F32, tag="mm", name=f"qps{fb}")
                for cb in range(FB):
                    for nh in range(2):
                        nc.tensor.matmul(
                            ps[:, nh * 512 : (nh + 1) * 512],
                            lhsT=wq_sb[:, cb, fb * 128 : (fb + 1) * 128],
                            rhs=qin_sb[:, cb, nh * 512 : (nh + 1) * 512],
                            start=(cb == 0),
                            stop=(cb == FB - 1),
                        )
                if evac_engine == "act":
                    nc.scalar.activation(qp_sb[:, fb, :], ps[:, :], AF.Identity)
                else:
                    nc.vector.tensor_copy(qp_sb[:, fb, :], ps[:, :])

            qproj(0, "act")

            # ---- v projection (interleaved into the first stream) ----------
            def vproj(tb):
                vch = vch_tiles[tb // 2]
                t0 = (tb % 2) * 128
                ps = (mmp if tb % 2 else ppp).tile(
                    [128, 1024], F32, tag="mm" if tb % 2 else "pp", name=f"vps{tb}")
                for n0, n1 in ((0, 512), (512, 768)):
                    for cb in range(FB):
                        nc.tensor.matmul(
                            ps[:, n0:n1],
                            lhsT=vch[:, cb, t0 : t0 + 128],
                            rhs=wv_sb[:, cb, n0:n1],
                            start=(cb == 0),
                            stop=(cb == FB - 1),
                        )
                nc.vector.tensor_copy(
                    vS3[:, tb, :, 0:64],
                    ps[:, 0:768].rearrange("p (h c) -> p h c", c=64),
                )

            # k projection; fb=0 up front, the rest in small interleaved chunks
            kps_tiles = {}

            def kproj_chunk(fb, tt, cbs, pool):
                if (fb, tt) not in kps_tiles:
                    kps_tiles[(fb, tt)] = pool.tile(
                        [128, 1024], F32, tag="pp" if pool is ppp else "mm",
                        name=f"kps{fb}_{tt}")
                ps = kps_tiles[(fb, tt)]
                for cb in cbs:
                    for nh in range(2):
                        o = tt * 1024 + nh * 512
                        nc.tensor.matmul(
                            ps[:, nh * 512 : (nh + 1) * 512],
                            lhsT=wk_sb[:, cb, fb * 128 : (fb + 1) * 128],
                            rhs=kin_sb[:, cb, o : o + 512],
                            start=(cb == 0),
                            stop=(cb == FB - 1),
                        )
                if cbs[-1] == FB - 1:
                    nc.vector.tensor_copy(kp_sb[:, fb, tt * 1024 : (tt + 1) * 1024], ps[:, :])
                    del kps_tiles[(fb, tt)]

            def kproj(fb, pool, tts=(0, 1)):
                for tt in tts:
                    kproj_chunk(fb, tt, list(range(FB)), pool)

            kproj(0, mmp)

            # ---- row-sum reciprocal groups (consolidated, approx-fast) -----
            def _prow(r):
                # head-row -> gather-tile partition (32-aligned group bases)
                return r if r < 6 else (26 + r if r < 10 else 54 + r)

            def recip_group(r0, r1, q0, q1):
                p0, p1 = _prow(r0), _prow(r1 - 1) + 1
                def g():
                    nq = q1 - q0
                    nc.vector.tensor_copy(rsF_sb[p0:p1, 0:nq], rsB_sb[p0:p1, q0:q1])
                    nc.vector.reciprocal(rsR_sb[p0:p1, 0:nq], rsF_sb[p0:p1, 0:nq])
                    nc.vector.tensor_copy(rsB_sb[p0:p1, q0:q1], rsR_sb[p0:p1, 0:nq])
                    nc.sync.dma_start(out=rrecip_dt[r0:r1, q0:q1], in_=rsB_sb[p0:p1, q0:q1])
                return [g]

            def norm_mul(hpi, q0, q1):
                def g():
                    nq = q1 - q0
                    rbb = rsp.tile([128, nq], BF16, tag="rbb", bufs=1, name=f"rbb{hpi}_{q0}")
                    nc.sync.dma_start(out=rbb[0:64, :], in_=_bcast_ap(rrecip_dt[2 * hpi : 2 * hpi + 1, q0:q1], 64))
                    nc.sync.dma_start(out=rbb[64:128, :], in_=_bcast_ap(rrecip_dt[2 * hpi + 1 : 2 * hpi + 2, q0:q1], 64))
                    nc.vector.tensor_mul(ctxS_sb[:, hpi, q0:q1], ctxS_sb[:, hpi, q0:q1], rbb[:, :])
                return [g]

            # ---- fc partials (emitted into the last attention stream) ------
            fcs = {}

            def fc_partial(qt):
                qsl = slice(qt * 128, (qt + 1) * 128)
                pool, tag = (ppp, "pp") if qt % 3 == 2 else (mmp, "mm")
                fcs[qt] = pool.tile([128, 1024], F32, tag=tag, name=f"fc{qt}")
                for hp in range(FB - 1):
                    for n0, n1 in ((0, 512), (512, 768)):
                        nc.tensor.matmul(
                            fcs[qt][:, n0:n1],
                            lhsT=ctxS_sb[:, hp, qsl],
                            rhs=wfc_sb[:, hp, n0:n1],
                            start=(hp == 0), stop=False,
                        )

            # ---- attention ---------------------------------------------------
            pending = []
            for hp in range(FB):
                for qh in range(2):
                    qof = qh * 512
                    c0 = ctxp.tile([65, 512], F32, tag="ctx", name=f"c0_{hp}_{qh}")
                    c1 = ctxp.tile([65, 512], F32, tag="ctx", name=f"c1_{hp}_{qh}")
                    for kb in range(KB):
                        ksl = slice(kb * 128, (kb + 1) * 128)
                        qsl = slice(qof, qof + 512)
                        sc = mmp.tile([128, 1024], F32, tag="mm", name=f"sc{hp}_{qh}_{kb}")
                        nc.tensor.matmul(
                            sc[:, 0:512], lhsT=kp_sb[0:64, hp, ksl], rhs=qp_sb[0:64, hp, qsl],
                            start=True, stop=True,
                        )
                        nc.tensor.matmul(
                            sc[:, 512:1024], lhsT=kp_sb[64:128, hp, ksl], rhs=qp_sb[64:128, hp, qsl],
                            start=True, stop=True,
                        )
                        # ---- interleaved filler (after scores, before ctx) --
                        if hp == 0 and qh == 0:
                            # all 16 v-projection tiles ride this stream; the
                            # 6-deep pt ring lets ctx lag while DMAs land
                            if kb % 2 == 0 and 4 + kb // 2 < 8:
                                vch_dma(4 + kb // 2)
                            vproj(kb)
                            if kb in (4, 8, 12):
                                qproj(1 + (4, 8, 12).index(kb), "dve")
                            if kb == KB - 1:
                                vchp.release()
                                inC.release()
                        if hp == 0 and qh == 1 and kb in (3, 7):
                            qproj(4 + (3, 7).index(kb), "dve")
                            if kb == 7:
                                inA.release()
                        if qh == 1 and hp < FB - 1 and kb in (2, 4, 6, 9, 11, 13):
                            i = (2, 4, 6, 9, 11, 13).index(kb)
                            kproj_chunk(hp + 1, i // 3, [2 * (i % 3), 2 * (i % 3) + 1], ppp)
                        if hp == FB - 1 and qh == 1 and kb in (2, 6, 10):
                            fc_partial((2, 6, 10).index(kb))
                        if pending and kb % 2 == 1:
                            pending.pop(0)()
                        # ---- exp -> mask -> context ------------------------
                        pt = ptp.tile([128, 1024], BF16, tag="pt", name=f"pt{hp}_{qh}_{kb}")
                        nc.scalar.activation(pt[:, :], sc[:, :], AF.Exp, scale=SCALE)
                        mk = mask_sb[:, kb, qsl]
                        nc.vector.tensor_mul(pt[:, 0:512], pt[:, 0:512], mk)
                        nc.vector.tensor_mul(pt[:, 512:1024], pt[:, 512:1024], mk)
                        nc.tensor.matmul(
                            c0[:, :], lhsT=vS3[:, kb, 2 * hp, 0:65], rhs=pt[:, 0:512],
                            start=(kb == 0), stop=(kb == KB - 1),
                        )
                        nc.tensor.matmul(
                            c1[:, :], lhsT=vS3[:, kb, 2 * hp + 1, 0:65], rhs=pt[:, 512:1024],
                            start=(kb == 0), stop=(kb == KB - 1),
                        )
                    # evacuate ctx + rowsum rows in one copy per head, then
                    # scatter via DMA (ctx to ctxS, rowsum row into the
                    # bf16 gather tile)
                    st0 = rsp.tile([65, 512], BF16, tag="st0")
                    nc.vector.tensor_copy(st0[:, :], c0[0:65, :])
                    nc.sync.dma_start(out=ctxS_sb[0:64, hp, qof : qof + 512], in_=st0[0:64, :])
                    p = _prow(2 * hp)
                    nc.sync.dma_start(out=rsB_sb[p : p + 1, qof : qof + 512], in_=st0[64:65, :])
                    st1 = rsp.tile([65, 512], BF16, tag="st1")
                    nc.vector.tensor_copy(st1[:, :], c1[0:65, :])
                    nc.sync.dma_start(out=ctxS_sb[64:128, hp, qof : qof + 512], in_=st1[0:64, :])
                    p = _prow(2 * hp + 1)
                    nc.sync.dma_start(out=rsB_sb[p : p + 1, qof : qof + 512], in_=st1[64:65, :])

                    # stagger reciprocal groups + normalizations into later
                    # streams (popped one per 2 kb)
                    if qh == 1 and hp == 2:
                        pending += recip_group(0, 6, 0, 512)
                        pending += recip_group(0, 6, 512, SQ)
                        for h in (0, 1, 2):
                            pending += norm_mul(h, 0, SQ)
                    elif qh == 1 and hp == 4:
                        pending += recip_group(6, 10, 0, 512)
                        pending += recip_group(6, 10, 512, SQ)
                        for h in (3, 4):
                            pending += norm_mul(h, 0, SQ)
                    elif qh == 0 and hp == FB - 1:
                        pending += recip_group(10, 12, 0, 512)
                        pending += norm_mul(5, 0, 512)
                    elif qh == 1 and hp == FB - 1:
                        for f in pending:
                            f()
                        for f in recip_group(10, 12, 512, SQ) + norm_mul(5, 512, SQ):
                            f()

            rsp.release()
            ptp.release()
            inB.release()

            # ---- fc + residual + LayerNorm ----------------------------------
            lnp = tc.alloc_tile_pool(name="ln", bufs=3)
            lns = tc.alloc_tile_pool(name="lnsmall", bufs=8)

            for qt in range(NQT):
                qsl = slice(qt * 128, (qt + 1) * 128)
                if qt not in fcs:
                    fc_partial(qt)
                fc = fcs.pop(qt)
                for n0, n1 in ((0, 512), (512, 768)):
                    nc.tensor.matmul(
                        fc[:, n0:n1],
                        lhsT=ctxS_sb[:, FB - 1, qsl],
                        rhs=wfc_sb[:, FB - 1, n0:n1],
                        start=False, stop=True,
                    )
                qr = lnp.tile([128, DM], F32, tag="qr")
                nc.sync.dma_start(out=qr[:], in_=qres_d[qsl, :])
                y = lnp.tile([128, DM], F32, tag="y")
                nc.vector.tensor_add(y[:], fc[:, 0:DM], qr[:])
                stats = lns.tile([128, 2, 6], F32, tag="stats")
                yr = y.rearrange("p (a b) -> p a b", a=2)
                nc.vector.bn_stats(out=stats[:, 0, :], in_=yr[:, 0, :])
                nc.vector.bn_stats(out=stats[:, 1, :], in_=yr[:, 1, :])
                mv = lns.tile([128, 2], F32, tag="mv")
                nc.vector.bn_aggr(out=mv[:], in_=stats[:])
                sd = lns.tile([128, 1], F32, tag="sd")
                nc.scalar.activation(sd[:], mv[:, 1:2], AF.Sqrt, bias=epsb[:])
                rstd = lns.tile([128, 1], F32, tag="rstd")
                nc.vector.reciprocal(rstd[:], sd[:])
                musr = lns.tile([128, 1], F32, tag="musr")
                nc.vector.tensor_scalar(
                    out=musr[:], in0=mv[:, 0:1], scalar1=rstd[:], scalar2=-1.0,
                    op0=ALU.mult, op1=ALU.mult,
                )
                o = lnp.tile([128, DM], F32, tag="o")
                nc.scalar.activation(o[:], y[:], AF.Identity, bias=musr[:], scale=rstd[:])
                nc.sync.dma_start(out=out_d[qsl, :], in_=o[:])

            lns.release()
            lnp.release()

    _split_excess_waits(nc)
    return nc


_NC_CACHE = None


def _get_nc():
    global _NC_CACHE
    if _NC_CACHE is None:
        _NC_CACHE = build_nc()
    return _NC_CACHE


def _prepare_in_maps(inputs):
    Q = np.asarray(inputs["Q"], np.float32)
    K = np.asarray(inputs["K"], np.float32)
    V = np.asarray(inputs["V"], np.float32)
    mask = np.asarray(inputs["attn_mask"])
    WQ = np.asarray(inputs["WQ"], np.float32)
    WK = np.asarray(inputs["WK"], np.float32)
    WV = np.asarray(inputs["WV"], np.float32)
    Wfc = np.asarray(inputs["Wfc"], np.float32)
    bQ = np.asarray(inputs["bQ"], np.float32)
    bK = np.asarray(inputs["bK"], np.float32)
    bV = np.asarray(inputs["bV"], np.float32)
    bfc = np.asarray(inputs["bfc"], np.float32)
    gamma = np.asarray(inputs["gamma"], np.float32)
    beta = np.asarray(inputs["beta"], np.float32)

    # the fast path skips the (identically-zero / identically-one) affine
    # terms that setup_inputs() produces; bfc folds into the residual
    if np.any(bQ) or np.any(bK) or np.any(bV) or np.any(gamma != 1.0) or np.any(beta):
        return None  # caller falls back to the numpy reference path

    bf = ml_dtypes.bfloat16
    wq = WQ.astype(bf)
    wk = WK.astype(bf)
    wv = WV.astype(bf)
    wfc = Wfc.astype(bf)

    keep = (~mask).astype(np.float32)
    in_maps = []
    for c in range(8):
        b, half = divmod(c, 2)
        qsl = slice(half * SQ, (half + 1) * SQ)
        in_maps.append(
            {
                "qT": np.ascontiguousarray(Q[b].T[:, qsl]).astype(bf),
                "kT": np.ascontiguousarray(K[b].T).astype(bf),
                "vT": np.ascontiguousarray(V[b].T).astype(bf),
                "maskT": np.ascontiguousarray(keep[b].T[:, qsl]).astype(bf),
                "wq": wq,
                "wk": wk,
                "wv": wv,
                "wfc": wfc,
                "qres": np.ascontiguousarray(Q[b, qsl, :] + bfc[None, :]),
            }
        )
    return in_maps


def _numpy_reference(inputs):
    """Escape hatch for input assumptions the device kernel doesn't cover."""
    Q = np.asarray(inputs["Q"], np.float32)
    K = np.asarray(inputs["K"], np.float32)
    V = np.asarray(inputs["V"], np.float32)
    mask = np.asarray(inputs["attn_mask"]).astype(bool)
    q = (Q @ inputs["WQ"] + inputs["bQ"]).reshape(B, S, H, DK).transpose(0, 2, 1, 3)
    k = (K @ inputs["WK"] + inputs["bK"]).reshape(B, S, H, DK).transpose(0, 2, 1, 3)
    v = (V @ inputs["WV"] + inputs["bV"]).reshape(B, S, H, DV).transpose(0, 2, 1, 3)
    out = np.empty((B, S, DM), np.float32)
    for b in range(B):
        for h in range(H):
            s = (q[b, h] @ k[b, h].T) / np.sqrt(DK)
            s = np.where(mask[b], np.float32(-1e9), s)
            s -= s.max(-1, keepdims=True)
            p = np.exp(s)
            p /= p.sum(-1, keepdims=True)
            ctx = p @ v[b, h]
            if h == 0:
                acc = np.zeros((S, DM), np.float32)
            acc += ctx @ np.asarray(inputs["Wfc"], np.float32)[h * DV : (h + 1) * DV, :]
        y = acc + inputs["bfc"][None, :] + Q[b]
        mu = y.mean(-1, keepdims=True)
        var = ((y - mu) ** 2).mean(-1, keepdims=True)
        out[b] = (y - mu) / np.sqrt(var + LN_EPS) * inputs["gamma"] + inputs["beta"]
    return out


def kernel(**inputs):
    in_maps = _prepare_in_maps(inputs)
    if in_maps is None:
        return _numpy_reference(inputs)
    nc = _get_nc()
    res = run_bass_kernel_spmd(nc, in_maps, list(range(8)))
    out = np.empty((B, S, DM), np.float32)
    for c in range(8):
        b, half = divmod(c, 2)
        out[b, half * SQ : (half + 1) * SQ, :] = res.results[c]["out"]
    return out


# revision 28
# speedup vs baseline: 1.1194x; 1.1194x over previous
"""Trainium2 Bass kernel for nn_MultiHeadAttention_52398601011223.

Full-input contract: kernel(**inputs) takes the complete tensors from
setup_inputs() and returns the full [4, 2048, 768] float32 output.

Sharding: 8 cores = batch(4) x query-half(2). Each core handles all 12
heads for 1024 queries of one batch, with all 2048 keys. No collectives:
each core owns its output rows end-to-end (k/v projections are computed
redundantly by the two cores sharing a batch).

Layout strategy (everything lands in its matmul-natural layout):
  - host pre-transposes Q/K/V to [768, seq] and pre-transposes the
    attention mask to a bf16 keep-mask [keys, queries]
  - projections produce qT/kT as [feature, token] (feature on partitions,
    2 heads per 128-partition block) and v as [token, feature]
  - scores are computed transposed, S^T[k, q], two heads row-packed in the
    128x128 array (d_k = 64); the pair runs concurrently via row tiling
  - exp on ScalarE (PSUM fp32 -> SBUF bf16), keep-mask applied
    multiplicatively on VectorE after exp (exp(-1e9) == 0 in the reference)
  - context uses lhsT = [V_head | ones] (65 columns): one accumulation
    yields both context^T and the softmax row-sums
  - row-sums are DMA-gathered straight out of the context PSUM rows and
    inverted with a few consolidated reciprocal_approx_fast ops
  - fc consumes context^T directly; residual + LayerNorm are per-token with
    d_model on the free axis

Scheduling: the exp stream on ScalarE (192 x ~1.1us) and the matmul
stream on TensorE are co-critical.  The front is restructured so the
first exp issues as early as possible (chunked weight DMAs so fb0 of the
q/k projections can start immediately); the q projection for fb1-5, the
whole v projection, the k projection for fb1-5 and the first fc partials
are all interleaved into the attention streams right after each score
matmul; VectorE work is trimmed (consolidated approx reciprocals, 2-op
mask multiply) to keep the scores->exp->mask->context chain tight.
"""

import numpy as np
import ml_dtypes

import concourse.bass as bass
import concourse.mybir as mybir
import concourse.tile as tile
import bass_rust
from concourse.bass_utils import run_bass_kernel_spmd

F32 = mybir.dt.float32
BF16 = mybir.dt.bfloat16
AF = mybir.ActivationFunctionType
ALU = mybir.AluOpType

B, S, DM = 4, 2048, 768
H, DK, DV = 12, 64, 64
SQ = S // 2          # queries per core
KB = S // 128        # key blocks (16)
FB = DM // 128       # feature blocks (6)
QT = SQ // 512       # 512-wide query tiles (2)
NQT = SQ // 128      # 128-row query tiles for fc/LN (8)
SCALE = 1.0 / 8.0    # 1/sqrt(d_k)
LN_EPS = 1e-5
VS = 66              # per-head stride in the v+ones sbuf layout


def _split_excess_waits(nc, maxw=1):
    """walrus CoreV3 in this build accepts only one sem-wait per
    instruction; move extras onto injected NoOps just before the owner."""
    n_new = 0
    for bb in nc.main_func.blocks:
        insts = bb.instructions  # live list
        i = 0
        while i < len(insts):
            ins = insts[i]
            si = getattr(ins, "sync_info", None)
            if si is None:
                i += 1
                continue
            waits = list(si.on_wait or [])
            if len(waits) > maxw:
                si.on_wait = waits[-maxw:]
                extra = waits[:-maxw]
                pos = i
                for j in range(0, len(extra), maxw):
                    nop = mybir.InstNoOp(name=f"waitsplit{n_new}", ins=[], outs=[])
                    n_new += 1
                    nop.engine = ins.engine
                    nop.sync_info = bass_rust.SyncInfo(
                        on_wait=extra[j : j + maxw], on_update=[]
                    )
                    insts.insert(pos, nop)
                    pos += 1
                    i += 1
            i += 1
    return n_new


def _bcast_ap(ap, nparts):
    """Partition-broadcast read AP over a [1, N] slice."""
    return bass.AP(tensor=ap.tensor, offset=ap.offset, ap=[[0, nparts]] + list(ap.ap[1:]))


def build_nc():
    nc = bass.Bass("TRN2", target_bir_lowering=False, debug=False, num_devices=8)

    qT_d = nc.dram_tensor("qT", [DM, SQ], BF16, kind="ExternalInput")
    kT_d = nc.dram_tensor("kT", [DM, S], BF16, kind="ExternalInput")
    vT_d = nc.dram_tensor("vT", [DM, S], BF16, kind="ExternalInput")
    maskT_d = nc.dram_tensor("maskT", [S, SQ], BF16, kind="ExternalInput")
    wq_d = nc.dram_tensor("wq", [DM, DM], BF16, kind="ExternalInput")
    wk_d = nc.dram_tensor("wk", [DM, DM], BF16, kind="ExternalInput")
    wv_d = nc.dram_tensor("wv", [DM, DM], BF16, kind="ExternalInput")
    wfc_d = nc.dram_tensor("wfc", [DM, DM], BF16, kind="ExternalInput")
    qres_d = nc.dram_tensor("qres", [SQ, DM], F32, kind="ExternalInput")
    out_d = nc.dram_tensor("out", [SQ, DM], F32, kind="ExternalOutput")

    wq_r = wq_d.ap().rearrange("(a p) f -> p a f", p=128)
    wk_r = wk_d.ap().rearrange("(a p) f -> p a f", p=128)
    mask_r = maskT_d.ap().rearrange("(a p) q -> p a q", p=128)

    with tile.TileContext(nc) as tc:
        with (
            tc.tile_pool(name="consts", bufs=1) as consts,
            tc.tile_pool(name="proj", bufs=1) as proj,
            tc.tile_pool(name="mm", bufs=2, space="PSUM") as mmp,
            tc.tile_pool(name="pp", bufs=1, space="PSUM") as ppp,
            tc.tile_pool(name="ctx", bufs=2, space="PSUM") as ctxp,
            tc.tile_pool(name="dram", bufs=2, space="DRAM") as dramp,
        ):
            # ---- persistent tiles -------------------------------------------
            wfc_sb = consts.tile([128, FB, DM], BF16, tag="wfc")
            mask_sb = consts.tile([128, KB, SQ], BF16, tag="mask")
            epsb = consts.tile([128, 1], F32, tag="epsb")
            nc.vector.memset(epsb[:], LN_EPS)
            warm = consts.tile([1, 2], F32, tag="warm")
            nc.vector.memset(warm[:], 1.0)
            # prewarm the exp table set while the front DMAs run
            nc.scalar.activation(warm[:, 1:2], warm[:, 0:1], AF.Exp)

            qp_sb = proj.tile([128, FB, SQ], BF16, tag="qp")
            kp_sb = proj.tile([128, FB, S], BF16, tag="kp")
            vS_sb = proj.tile([128, KB, H * VS], BF16, tag="vS")
            ctxS_sb = proj.tile([128, FB, SQ], BF16, tag="ctxS")
            # row-sum path: the bf16 rowsum rows are DMA-scattered into a
            # partition-PACKED layout (head,qh,128-chunk -> partition) so the
            # consolidated reciprocals run at FD=128.  DVE partition bases
            # must be 32-aligned: head groups 0-5 / 6-9 / 10-11 sit at
            # partition bases 0 / 64 / 96.
            rsB_sb = proj.tile([128, 128], BF16, tag="rsB")
            rsF_sb = proj.tile([128, 128], F32, tag="rsF")
            rsR_sb = proj.tile([128, 128], F32, tag="rsR")
            rsBo_sb = proj.tile([128, 128], BF16, tag="rsBo")
            rrecip_dt = dramp.tile([H, SQ], BF16, tag="rrecip")

            vS3 = vS_sb.rearrange("p b (h c) -> p b h c", c=VS)
            for tb in range(KB):
                nc.vector.memset(vS3[:, tb, :, 64:66], 1.0)

            # ---- pools (alloc order = reverse of release order: LIFO) ------
            inB = tc.alloc_tile_pool(name="inB", bufs=1)
            wk_sb = inB.tile([128, FB, DM], BF16, tag="wk")
            kin_sb = inB.tile([128, FB, S], BF16, tag="kin")
            ptp = tc.alloc_tile_pool(name="pt", bufs=5)
            rsp = tc.alloc_tile_pool(name="rs", bufs=2)
            inA = tc.alloc_tile_pool(name="inA", bufs=1)
            wq_sb = inA.tile([128, FB, DM], BF16, tag="wq")
            qin_sb = inA.tile([128, FB, SQ], BF16, tag="qin")
            inC = tc.alloc_tile_pool(name="inC", bufs=1)
            wv_sb = inC.tile([128, FB, DM], BF16, tag="wv")
            vchp = tc.alloc_tile_pool(name="vch", bufs=3)

            # ---- front DMAs: ordered so fb0 q/k-proj can start ASAP ---------
            vT_r = vT_d.ap().rearrange("(a p) t -> p a t", p=128)
            vch_tiles = {}

            def vch_dma(c):
                vch = vchp.tile([128, FB, 256], BF16, tag="vch", name=f"vch{c}")
                vch_tiles[c] = vch
                nc.sync.dma_start(out=vch[:], in_=vT_r[:, :, c * 256 : (c + 1) * 256])

            # minimal set gating the first score matmul, then the rest
            nc.sync.dma_start(out=wq_sb[:, :, 0:128], in_=wq_r[:, :, 0:128])
            nc.sync.dma_start(out=qin_sb[:], in_=qT_d.ap().rearrange("(a p) t -> p a t", p=128))
            nc.sync.dma_start(out=wk_sb[:, :, 0:128], in_=wk_r[:, :, 0:128])
            nc.sync.dma_start(out=kin_sb[:], in_=kT_d.ap().rearrange("(a p) t -> p a t", p=128))
            nc.sync.dma_start(out=wq_sb[:, :, 128:768], in_=wq_r[:, :, 128:768])
            nc.sync.dma_start(out=mask_sb[:, 0:4, :], in_=mask_r[:, 0:4, :])
            nc.sync.dma_start(out=wv_sb[:], in_=wv_d.ap().rearrange("(a p) f -> p a f", p=128))
            vch_dma(0)
            vch_dma(1)
            nc.sync.dma_start(out=mask_sb[:, 4:8, :], in_=mask_r[:, 4:8, :])
            vch_dma(2)
            vch_dma(3)
            nc.sync.dma_start(out=wk_sb[:, :, 128:768], in_=wk_r[:, :, 128:768])
            nc.sync.dma_start(out=mask_sb[:, 8:KB, :], in_=mask_r[:, 8:KB, :])
            nc.sync.dma_start(out=wfc_sb[:], in_=wfc_d.ap().rearrange("(a p) f -> p a f", p=128))

            # ---- q projection: fb0 up front (ScalarE evac: ACT is idle), ----
            # fb1-5 interleaved into the attention streams as whole blocks
            def qproj(fb, evac_engine):
                ps = mmp.tile([128, 1024], F32, tag="mm", name=f"qps{fb}")
                for cb in range(FB):
                    for nh in range(2):
                        nc.tensor.matmul(
                            ps[:, nh * 512 : (nh + 1) * 512],
                            lhsT=wq_sb[:, cb, fb * 128 : (fb + 1) * 128],
                            rhs=qin_sb[:, cb, nh * 512 : (nh + 1) * 512],
                            start=(cb == 0),
                            stop=(cb == FB - 1),
                        )
                if evac_engine == "act":
                    nc.scalar.activation(qp_sb[:, fb, :], ps[:, :], AF.Identity)
                else:
                    nc.vector.tensor_copy(qp_sb[:, fb, :], ps[:, :])

            qproj(0, "act")

            # ---- v projection (interleaved into the first stream) ----------
            def vproj(tb):
                vch = vch_tiles[tb // 2]
                t0 = (tb % 2) * 128
                ps = (mmp if tb % 2 else ppp).tile(
                    [128, 1024], F32, tag="mm" if tb % 2 else "pp", name=f"vps{tb}")
                for n0, n1 in ((0, 512), (512, 768)):
                    for cb in range(FB):
                        nc.tensor.matmul(
                            ps[:, n0:n1],
                            lhsT=vch[:, cb, t0 : t0 + 128],
                            rhs=wv_sb[:, cb, n0:n1],
                            start=(cb == 0),
                            stop=(cb == FB - 1),
                        )
                nc.vector.tensor_copy(
                    vS3[:, tb, :, 0:64],
                    ps[:, 0:768].rearrange("p (h c) -> p h c", c=64),
                )

            # k projection; fb=0 up front, the rest in small interleaved chunks
            kps_tiles = {}

            def kproj_chunk(fb, tt, cbs, pool):
                if (fb, tt) not in kps_tiles:
                    kps_tiles[(fb, tt)] = pool.tile(
                        [128, 1024], F32, tag="pp" if pool is ppp else "mm",
                        name=f"kps{fb}_{tt}")
                ps = kps_tiles[(fb, tt)]
                for cb in cbs:
                    for nh in range(2):
                        o = tt * 1024 + nh * 512
                        nc.tensor.matmul(
                            ps[:, nh * 512 : (nh + 1) * 512],
                            lhsT=wk_sb[:, cb, fb * 128 : (fb + 1) * 128],
                            rhs=kin_sb[:, cb, o : o + 512],
                            start=(cb == 0),
                            stop=(cb == FB - 1),
                        )
                if cbs[-1] == FB - 1:
                    nc.vector.tensor_copy(kp_sb[:, fb, tt * 1024 : (tt + 1) * 1024], ps[:, :])
                    del kps_tiles[(fb, tt)]

            def kproj(fb, pool, tts=(0, 1)):
                for tt in tts:
                    kproj_chunk(fb, tt, list(range(FB)), pool)

            kproj(0, mmp)

            # ---- row-sum reciprocal groups (consolidated, FD=128) ----------
            def _prow(r, qh):
                # (head-row, qh) -> packed base partition (4 chunk parts)
                base = 8 * r if r < 6 else (16 + 8 * r if r < 10 else 16 + 8 * r)
                return base + 4 * qh

            def recip_group(r0, r1):
                p0 = _prow(r0, 0)
                p1 = _prow(r1 - 1, 1) + 4
                def g():
                    nc.vector.tensor_copy(rsF_sb[p0:p1, :], rsB_sb[p0:p1, :])
                    nc.vector.reciprocal(rsR_sb[p0:p1, :], rsF_sb[p0:p1, :])
                    nc.vector.tensor_copy(rsBo_sb[p0:p1, :], rsR_sb[p0:p1, :])
                    for r in range(r0, r1):
                        nc.sync.dma_start(
                            out=rrecip_dt[r : r + 1, :],
                            in_=rsBo_sb[_prow(r, 0) : _prow(r, 0) + 8, :],
                        )
                return [g]

            def norm_mul(hpi, q0, q1):
                def g():
                    nq = q1 - q0
                    rbb = rsp.tile([128, nq], BF16, tag="rbb", bufs=1, name=f"rbb{hpi}_{q0}")
                    nc.sync.dma_start(out=rbb[0:64, :], in_=_bcast_ap(rrecip_dt[2 * hpi : 2 * hpi + 1, q0:q1], 64))
                    nc.sync.dma_start(out=rbb[64:128, :], in_=_bcast_ap(rrecip_dt[2 * hpi + 1 : 2 * hpi + 2, q0:q1], 64))
                    nc.vector.tensor_mul(ctxS_sb[:, hpi, q0:q1], ctxS_sb[:, hpi, q0:q1], rbb[:, :])
                return [g]

            # ---- fc partials (emitted into the last attention stream) ------
            fcs = {}

            def fc_partial(qt):
                qsl = slice(qt * 128, (qt + 1) * 128)
                pool, tag = (ppp, "pp") if qt % 3 == 2 else (mmp, "mm")
                fcs[qt] = pool.tile([128, 1024], F32, tag=tag, name=f"fc{qt}")
                for hp in range(FB - 1):
                    for n0, n1 in ((0, 512), (512, 768)):
                        nc.tensor.matmul(
                            fcs[qt][:, n0:n1],
                            lhsT=ctxS_sb[:, hp, qsl],
                            rhs=wfc_sb[:, hp, n0:n1],
                            start=(hp == 0), stop=False,
                        )

            # ---- attention ---------------------------------------------------
            pending = []
            for hp in range(FB):
                for qh in range(2):
                    qof = qh * 512
                    c0 = ctxp.tile([65, 512], F32, tag="ctx", name=f"c0_{hp}_{qh}")
                    c1 = ctxp.tile([65, 512], F32, tag="ctx", name=f"c1_{hp}_{qh}")
                    for kb in range(KB):
                        ksl = slice(kb * 128, (kb + 1) * 128)
                        qsl = slice(qof, qof + 512)
                        sc = mmp.tile([128, 1024], F32, tag="mm", name=f"sc{hp}_{qh}_{kb}")
                        nc.tensor.matmul(
                            sc[:, 0:512], lhsT=kp_sb[0:64, hp, ksl], rhs=qp_sb[0:64, hp, qsl],
                            start=True, stop=True,
                        )
                        nc.tensor.matmul(
                            sc[:, 512:1024], lhsT=kp_sb[64:128, hp, ksl], rhs=qp_sb[64:128, hp, qsl],
                            start=True, stop=True,
                        )
                        # ---- interleaved filler (after scores, before ctx) --
                        if hp == 0 and qh == 0:
                            # all 16 v-projection tiles ride this stream; the
                            # 5-deep pt ring lets ctx lag while DMAs land
                            if kb % 2 == 0 and 4 + kb // 2 < 8:
                                vch_dma(4 + kb // 2)
                            vproj(kb)
                            if kb in (8, 12):
                                qproj(1 + (8, 12).index(kb), "dve")
                            if kb == KB - 1:
                                vchp.release()
                                inC.release()
                        if hp == 0 and qh == 1 and kb in (1, 5, 9):
                            qproj(3 + (1, 5, 9).index(kb), "dve")
                            if kb == 9:
                                inA.release()
                        if qh == 1 and hp < FB - 1 and kb in (2, 4, 6, 9, 11, 13):
                            i = (2, 4, 6, 9, 11, 13).index(kb)
                            kproj_chunk(hp + 1, i // 3, [2 * (i % 3), 2 * (i % 3) + 1], ppp)
                        if pending and kb % 2 == 1:
                            pending.pop(0)()
                        # ---- exp -> mask -> context ------------------------
                        pt = ptp.tile([128, 1024], BF16, tag="pt", name=f"pt{hp}_{qh}_{kb}")
                        nc.scalar.activation(pt[:, :], sc[:, :], AF.Exp, scale=SCALE)
                        mk = mask_sb[:, kb, qsl]
                        mk2 = bass.AP(tensor=mk.tensor, offset=mk.offset,
                                      ap=[mk.ap[0], [0, 2]] + list(mk.ap[1:]))
                        nc.vector.tensor_mul(pt[:, :].rearrange("p (a b) -> p a b", a=2), pt[:, :].rearrange("p (a b) -> p a b", a=2), mk2)
                        nc.tensor.matmul(
                            c0[:, :], lhsT=vS3[:, kb, 2 * hp, 0:65], rhs=pt[:, 0:512],
                            start=(kb == 0), stop=(kb == KB - 1),
                        )
                        nc.tensor.matmul(
                            c1[:, :], lhsT=vS3[:, kb, 2 * hp + 1, 0:65], rhs=pt[:, 512:1024],
                            start=(kb == 0), stop=(kb == KB - 1),
                        )
                    # evacuate ctx + rowsum rows in one copy per head, then
                    # scatter via DMA (ctx to ctxS, rowsum row into the
                    # bf16 gather tile)
                    st0 = rsp.tile([65, 512], BF16, tag="st0")
                    nc.vector.tensor_copy(st0[:, :], c0[0:65, :])
                    nc.sync.dma_start(out=ctxS_sb[0:64, hp, qof : qof + 512], in_=st0[0:64, :])
                    p = _prow(2 * hp, qh)
                    nc.sync.dma_start(out=rsB_sb[p : p + 4, :], in_=st0[64:65, :])
                    st1 = rsp.tile([65, 512], BF16, tag="st1")
                    nc.vector.tensor_copy(st1[:, :], c1[0:65, :])
                    nc.sync.dma_start(out=ctxS_sb[64:128, hp, qof : qof + 512], in_=st1[0:64, :])
                    p = _prow(2 * hp + 1, qh)
                    nc.sync.dma_start(out=rsB_sb[p : p + 4, :], in_=st1[64:65, :])

                    # stagger reciprocal groups + normalizations into later
                    # streams (popped one per 2 kb)
                    if qh == 1 and hp == 2:
                        pending += recip_group(0, 6)
                        for h in (0, 1, 2):
                            pending += norm_mul(h, 0, SQ)
                    elif qh == 1 and hp == 4:
                        pending += recip_group(6, 10)
                        for h in (3, 4):
                            pending += norm_mul(h, 0, SQ)
                    elif qh == 1 and hp == FB - 1:
                        for f in pending:
                            f()
                        for f in recip_group(10, 12) + norm_mul(5, 0, SQ):
                            f()

            rsp.release()
            ptp.release()
            inB.release()

            # ---- fc + residual + LayerNorm ----------------------------------
            lnp = tc.alloc_tile_pool(name="ln", bufs=3)
            lns = tc.alloc_tile_pool(name="lnsmall", bufs=8)

            for qt in range(3):
                fc_partial(qt)
            for qt in range(NQT):
                qsl = slice(qt * 128, (qt + 1) * 128)
                if qt not in fcs:
                    fc_partial(qt)
                fc = fcs.pop(qt)
                for n0, n1 in ((0, 512), (512, 768)):
                    nc.tensor.matmul(
                        fc[:, n0:n1],
                        lhsT=ctxS_sb[:, FB - 1, qsl],
                        rhs=wfc_sb[:, FB - 1, n0:n1],
                        start=False, stop=True,
                    )
                qr = lnp.tile([128, DM], F32, tag="qr")
                nc.sync.dma_start(out=qr[:], in_=qres_d[qsl, :])
                y = lnp.tile([128, DM], F32, tag="y")
                nc.vector.tensor_add(y[:], fc[:, 0:DM], qr[:])
                stats = lns.tile([128, 2, 6], F32, tag="stats")
                yr = y.rearrange("p (a b) -> p a b", a=2)
                nc.vector.bn_stats(out=stats[:, 0, :], in_=yr[:, 0, :])
                nc.vector.bn_stats(out=stats[:, 1, :], in_=yr[:, 1, :])
                mv = lns.tile([128, 2], F32, tag="mv")
                nc.vector.bn_aggr(out=mv[:], in_=stats[:])
                sd = lns.tile([128, 1], F32, tag="sd")
                nc.scalar.activation(sd[:], mv[:, 1:2], AF.Sqrt, bias=epsb[:])
                rstd = lns.tile([128, 1], F32, tag="rstd")
                nc.vector.reciprocal(rstd[:], sd[:])
                musr = lns.tile([128, 1], F32, tag="musr")
                nc.vector.tensor_scalar(
                    out=musr[:], in0=mv[:, 0:1], scalar1=rstd[:], scalar2=-1.0,
                    op0=ALU.mult, op1=ALU.mult,
                )
                o = lnp.tile([128, DM], F32, tag="o")
                nc.scalar.activation(o[:], y[:], AF.Identity, bias=musr[:], scale=rstd[:])
                nc.sync.dma_start(out=out_d[qsl, :], in_=o[:])

            lns.release()
            lnp.release()

    _split_excess_waits(nc)
    return nc


_NC_CACHE = None


def _get_nc():
    global _NC_CACHE
    if _NC_CACHE is None:
        _NC_CACHE = build_nc()
    return _NC_CACHE


def _prepare_in_maps(inputs):
    Q = np.asarray(inputs["Q"], np.float32)
    K = np.asarray(inputs["K"], np.float32)
    V = np.asarray(inputs["V"], np.float32)
    mask = np.asarray(inputs["attn_mask"])
    WQ = np.asarray(inputs["WQ"], np.float32)
    WK = np.asarray(inputs["WK"], np.float32)
    WV = np.asarray(inputs["WV"], np.float32)
    Wfc = np.asarray(inputs["Wfc"], np.float32)
    bQ = np.asarray(inputs["bQ"], np.float32)
    bK = np.asarray(inputs["bK"], np.float32)
    bV = np.asarray(inputs["bV"], np.float32)
    bfc = np.asarray(inputs["bfc"], np.float32)
    gamma = np.asarray(inputs["gamma"], np.float32)
    beta = np.asarray(inputs["beta"], np.float32)

    # the fast path skips the (identically-zero / identically-one) affine
    # terms that setup_inputs() produces; bfc folds into the residual
    if np.any(bQ) or np.any(bK) or np.any(bV) or np.any(gamma != 1.0) or np.any(beta):
        return None  # caller falls back to the numpy reference path

    bf = ml_dtypes.bfloat16
    wq = WQ.astype(bf)
    wk = WK.astype(bf)
    wv = WV.astype(bf)
    wfc = Wfc.astype(bf)

    keep = (~mask).astype(np.float32)
    in_maps = []
    for c in range(8):
        b, half = divmod(c, 2)
        qsl = slice(half * SQ, (half + 1) * SQ)
        in_maps.append(
            {
                "qT": np.ascontiguousarray(Q[b].T[:, qsl]).astype(bf),
                "kT": np.ascontiguousarray(K[b].T).astype(bf),
                "vT": np.ascontiguousarray(V[b].T).astype(bf),
                "maskT": np.ascontiguousarray(keep[b].T[:, qsl]).astype(bf),
                "wq": wq,
                "wk": wk,
                "wv": wv,
                "wfc": wfc,
                "qres": np.ascontiguousarray(Q[b, qsl, :] + bfc[None, :]),
            }
        )
    return in_maps


def _numpy_reference(inputs):
    """Escape hatch for input assumptions the device kernel doesn't cover."""
    Q = np.asarray(inputs["Q"], np.float32)
    K = np.asarray(inputs["K"], np.float32)
    V = np.asarray(inputs["V"], np.float32)
    mask = np.asarray(inputs["attn_mask"]).astype(bool)
    q = (Q @ inputs["WQ"] + inputs["bQ"]).reshape(B, S, H, DK).transpose(0, 2, 1, 3)
    k = (K @ inputs["WK"] + inputs["bK"]).reshape(B, S, H, DK).transpose(0, 2, 1, 3)
    v = (V @ inputs["WV"] + inputs["bV"]).reshape(B, S, H, DV).transpose(0, 2, 1, 3)
    out = np.empty((B, S, DM), np.float32)
    for b in range(B):
        for h in range(H):
            s = (q[b, h] @ k[b, h].T) / np.sqrt(DK)
            s = np.where(mask[b], np.float32(-1e9), s)
            s -= s.max(-1, keepdims=True)
            p = np.exp(s)
            p /= p.sum(-1, keepdims=True)
            ctx = p @ v[b, h]
            if h == 0:
                acc = np.zeros((S, DM), np.float32)
            acc += ctx @ np.asarray(inputs["Wfc"], np.float32)[h * DV : (h + 1) * DV, :]
        y = acc + inputs["bfc"][None, :] + Q[b]
        mu = y.mean(-1, keepdims=True)
        var = ((y - mu) ** 2).mean(-1, keepdims=True)
        out[b] = (y - mu) / np.sqrt(var + LN_EPS) * inputs["gamma"] + inputs["beta"]
    return out


def kernel(**inputs):
    in_maps = _prepare_in_maps(inputs)
    if in_maps is None:
        return _numpy_reference(inputs)
    nc = _get_nc()
    res = run_bass_kernel_spmd(nc, in_maps, list(range(8)))
    out = np.empty((B, S, DM), np.float32)
    for c in range(8):
        b, half = divmod(c, 2)
        out[b, half * SQ : (half + 1) * SQ, :] = res.results[c]["out"]
    return out
